# revision 24
# baseline (speedup 1.0000x reference)
"""Multi-head cross-modal attention + residual + LayerNorm on 8 TRN2 cores.

Reference computation (per batch b):
  Q = query @ Wq.T + bq ; K = key @ Wk.T + bk ; V = value @ Wv.T + bv
  attn = softmax(Q K^T / sqrt(D)) per head
  out  = (attn V) @ Wo.T + bo
  y    = LayerNorm(out + query) * gamma + beta

Sharding: 2-D over (batch=4) x (query-half=2). Core c owns batch c//2 and
queries [512*(c%2), 512*(c%2)+512); it computes ALL heads for its query
rows (K/V projections are duplicated across the pair of cores sharing a
batch), so there is NO collective: each core emits its own 512 rows of
the final LayerNorm output.

Precision/PE strategy: EVERY matmul (projections, scores, attn@V,
out-proj) runs fp8 with MatmulPerfMode.DoubleRow at 0.5 cycles/row.
Scores use a zero-slot DR trick: the per-head contraction is only 64, so
the DR pair's second slot is nulled by keeping a zeroed second slot in
the Q operand (QT8[:, t, 1, :] == 0); the K operand's second slot then
reads the next key block (or a zeroed pad column for jt=15) and
contributes exactly 0. Weights are pre-scaled x32 on host so fp8 sees
~N(0,1); the rescale folds into the PSUM->SBUF drain instructions.

bk is dropped entirely: K = Wk k + bk shifts every score of a given
query by the same constant (Q.bk), and softmax over keys is invariant to
per-query shifts. bv is folded on host into the residual (+ bv @ Wo.T).

Softmax: no max-subtraction (|score/8| < 7, fp32/e5m2 exp cannot
overflow). exp is split ACT/DVE per pair: ACT runs hardware Exp -> e5m2;
DVE runs a Schraudolph bit trick (round(a*score + b) written as int8,
bit-identical to e5m2) whose systematic error cancels in the softmax
ratio. Per-head normalization: DVE reciprocal of the ones-column
denominator row, Pool partition_broadcast down 64 partitions, one DVE
scalar_tensor_tensor into the fp8 ctx tile (x8 fp8 scale).

Drains (PSUM->SBUF) are batched two PSUM banks per instruction where no
per-partition bias is needed (K and V projections), and all memsets run
on the otherwise-idle Pool engine. DMA is issued in need-order so the
head pipeline starts as soon as the first K chunk lands.
"""

import sys

if "/opt/trn_rl_repo" not in sys.path:
    sys.path.insert(0, "/opt/trn_rl_repo")

import ml_dtypes
import numpy as np

import concourse.bass as bass  # noqa: F401  (registers types)
import concourse.mybir as mybir
import concourse.tile as tile
from concourse import bacc
from concourse.bass_utils import run_bass_kernel_spmd

F32 = mybir.dt.float32
BF16 = mybir.dt.bfloat16
F8E4 = mybir.dt.float8e4
F8E5 = mybir.dt.float8e5
I8 = mybir.dt.int8
AF = mybir.ActivationFunctionType
OP = mybir.AluOpType
DR = mybir.MatmulPerfMode.DoubleRow

B, SQ, SK, E, H, D = 4, 1024, 2048, 1024, 16, 64
N_CORES = 8
QR = 512           # queries per core
EPS = 1e-5
WSCALE = 32.0      # host-side weight pre-scale for fp8
CTXSCALE = 8.0     # fp8 scale applied to normalized ctx

# Schraudolph exp -> e5m2 bits: bits = trunc(EXPA * raw_score + EXPB)
EXPA = 0.125 * 4.0 / float(np.log(2.0))
EXPB = 58.5

# exp engine split per (head, jt-pair): A=ACT hw exp, D=DVE bit-trick.
# 64 A / 64 D overall; heads 0-1 are DVE-heavy (ACT carries the prologue
# drains while DMA streams in), heads 12-15 are ACT-heavy (DVE freed for
# the epilogue reciprocal/stt/LN chains).
PATTERNS = [
    "DDDDDDDD",  # h0
    "DDADDADA",  # h1
    "ADADADAD",  # h2
    "ADADADAD",  # h3
    "ADADADAD",  # h4
    "ADADADAD",  # h5
    "ADADADAD",  # h6
    "ADADADAD",  # h7
    "ADADADAD",  # h8
    "ADADADAD",  # h9
    "ADADADAD",  # h10
    "ADADADAD",  # h11
    "ADADADAD",  # h12
    "ADADADAD",  # h13
    "AADAADAA",  # h14
    "AADAADAA",  # h15
]

# module-level knobs used by test.py (harness ignores them)
TRACE = False
LAST_RESULTS = None

_NC_CACHE = None


def _build_nc():
    nc = bacc.Bacc(None, target_bir_lowering=False)

    q8 = nc.dram_tensor("q8", [128, 8 * QR], F8E4, kind="ExternalInput")
    k8 = nc.dram_tensor("k8", [128, 8 * SK], F8E4, kind="ExternalInput")
    v8 = nc.dram_tensor("v8", [128, 16 * 8 * 128], F8E4, kind="ExternalInput")
    wq8 = nc.dram_tensor("wq8", [128, 8 * E], F8E4, kind="ExternalInput")
    wk8 = nc.dram_tensor("wk8", [128, 8 * E], F8E4, kind="ExternalInput")
    wv8 = nc.dram_tensor("wv8", [128, 8 * E], F8E4, kind="ExternalInput")
    wo8 = nc.dram_tensor("wo8", [128, 8 * E], F8E4, kind="ExternalInput")
    bq8 = nc.dram_tensor("bq8", [128, 8], F32, kind="ExternalInput")
    cs8 = nc.dram_tensor("cs8", [128, 8], F8E4, kind="ExternalInput")
    rs4 = nc.dram_tensor("rs4", [128, 4], BF16, kind="ExternalInput")
    resid = nc.dram_tensor("resid", [QR, E], BF16, kind="ExternalInput")
    vec3 = nc.dram_tensor("vec3", [2, E], BF16, kind="ExternalInput")
    ident = nc.dram_tensor("ident", [128, 128], BF16, kind="ExternalInput")
    out = nc.dram_tensor("out", [QR, E], BF16, kind="ExternalOutput")

    from contextlib import ExitStack

    with ExitStack() as ctx:
        tc = ctx.enter_context(tile.TileContext(nc))
        constp = ctx.enter_context(tc.tile_pool(name="consts", bufs=1))
        inp = ctx.enter_context(tc.tile_pool(name="inp", bufs=1))
        qt8p = ctx.enter_context(tc.tile_pool(name="qt8p", bufs=1))
        kt8p = ctx.enter_context(tc.tile_pool(name="kt8p", bufs=1))
        vsb = ctx.enter_context(tc.tile_pool(name="vsb", bufs=8))
        expp = ctx.enter_context(tc.tile_pool(name="expp", bufs=4))
        ctxp = ctx.enter_context(tc.tile_pool(name="ctxp", bufs=1))
        recp = ctx.enter_context(tc.tile_pool(name="recp", bufs=2))
        lnp = ctx.enter_context(tc.tile_pool(name="lnp", bufs=2))
        # one 6-bank pool serves scores pairs, projection groups (both
        # slots) and the out-proj epilogue; pc accumulators get 2 banks.
        psc = ctx.enter_context(tc.tile_pool(name="psc", bufs=3, space="PSUM"))
        pcp = ctx.enter_context(tc.tile_pool(name="pcp", bufs=2, space="PSUM"))
        if True:
            # ---------------- input DMAs (need-order) ----------------
            bq_sb = constp.tile([128, 8], F32)
            nc.sync.dma_start(out=bq_sb, in_=bq8[:, :])
            wq_t = inp.tile([128, 8 * E], F8E4, tag="wq")
            nc.sync.dma_start(out=wq_t, in_=wq8[:, :])
            q_t = inp.tile([128, 8 * QR], F8E4, tag="q")
            nc.sync.dma_start(out=q_t, in_=q8[:, :])
            wk_t = inp.tile([128, 8 * E], F8E4, tag="wk")
            nc.sync.dma_start(out=wk_t, in_=wk8[:, :])
            k_t = inp.tile([128, 8 * SK], F8E4, tag="k")
            kvd = k_t.rearrange("p (s c) -> p s c", s=8)
            k8d = k8[:, :].rearrange("p (s c) -> p s c", s=8)
            for jc in range(2):
                nc.sync.dma_start(
                    out=kvd[:, :, jc * 512 : (jc + 1) * 512],
                    in_=k8d[:, :, jc * 512 : (jc + 1) * 512],
                )
            wv_t = inp.tile([128, 8 * E], F8E4, tag="wv")
            nc.sync.dma_start(out=wv_t, in_=wv8[:, :])
            v_t = inp.tile([128, 16 * 8 * 128], F8E4, tag="v")
            vvd = v_t.rearrange("p (jb r) -> p jb r", jb=16)
            v8d = v8[:, :].rearrange("p (jb r) -> p jb r", jb=16)
            for jp in range(8):
                nc.sync.dma_start(
                    out=vvd[:, 2 * jp : 2 * jp + 2, :],
                    in_=v8d[:, 2 * jp : 2 * jp + 2, :],
                )
            for jc in range(2, 4):
                nc.sync.dma_start(
                    out=kvd[:, :, jc * 512 : (jc + 1) * 512],
                    in_=k8d[:, :, jc * 512 : (jc + 1) * 512],
                )

            # slot views: (partition, slot, col)
            qv = q_t.rearrange("p (s c) -> p s c", s=8)
            kv = k_t.rearrange("p (s c) -> p s c", s=8)
            vv = v_t.rearrange("p (jb s c) -> p jb s c", jb=16, s=8)
            wqv = wq_t.rearrange("p (s c) -> p s c", s=8)
            wkv = wk_t.rearrange("p (s c) -> p s c", s=8)
            wvv = wv_t.rearrange("p (s c) -> p s c", s=8)

            # ------------- fp8 operand tiles for the head loop -------------
            # QT8: [128 d, 8 t, 2 slot, 512 q]; slot 1 stays 0 so the DR
            # pair's second contraction tile contributes nothing.
            QT8 = qt8p.tile([128, 8, 2, QR], F8E4)
            nc.gpsimd.memset(QT8[:, :, 1, :], 0.0)
            # KT8: [128 d, 8 t, 16 jt, 128 k]. The scores lhsT presents
            # each jt block twice via a stride-0 axis (broadcast_to), so
            # the DR pair's second slot re-reads the same block and is
            # nulled by QT8's zero slot — no pad, no cross-chunk reads.
            KT8 = kt8p.tile([128, 8, 16, 128], F8E4)
            # V8 tiles: per jt-pair [128 j, 2 slot, 16 head, 65] fp8e4
            v8t = [
                vsb.tile([128, 2, H, 65], F8E4, tag="v8", name=f"V8_{pr}")
                for pr in range(8)
            ]
            for pr in range(8):
                nc.gpsimd.memset(v8t[pr][:, :, :, 64:65], 1.0)

            # ---------------- projection emitters ----------------
            def emit_qpair(tp):
                """Q proj for tiles t=2tp, 2tp+1 -> QT8 slot 0 (fp8)."""
                sp = psc.tile([128, 2, QR], F32, tag="sc", name=f"qp_{tp}")
                for i in range(2):
                    t = 2 * tp + i
                    for p in range(4):
                        nc.tensor.matmul(
                            sp[:, i, :],
                            wqv[:, 2 * p : 2 * p + 2, t * 128 : (t + 1) * 128],
                            qv[:, 2 * p : 2 * p + 2, :],
                            start=(p == 0),
                            stop=(p == 3),
                            perf_mode=DR,
                        )
                for i in range(2):
                    t = 2 * tp + i
                    nc.scalar.activation(
                        out=QT8[:, t, 0, :],
                        in_=sp[:, i, :],
                        func=AF.Identity,
                        bias=bq_sb[:, t : t + 1],
                        scale=1.0 / WSCALE,
                    )

            def emit_khalf(t, half):
                """K proj tile t, key chunks jc=2*half,2*half+1 -> KT8."""
                sp = psc.tile([128, 2, QR], F32, tag="sc", name=f"kh_{t}_{half}")
                for i in range(2):
                    jc = 2 * half + i
                    for p in range(4):
                        nc.tensor.matmul(
                            sp[:, i, :],
                            wkv[:, 2 * p : 2 * p + 2, t * 128 : (t + 1) * 128],
                            kv[:, 2 * p : 2 * p + 2, jc * 512 : (jc + 1) * 512],
                            start=(p == 0),
                            stop=(p == 3),
                            perf_mode=DR,
                        )
                nc.scalar.activation(
                    out=KT8[:, t, 8 * half : 8 * half + 8, :],
                    in_=sp[:],
                    func=AF.Copy,
                    scale=1.0 / WSCALE,
                )

            def emit_kchunk(t, jc):
                """K proj tile t, single key chunk jc -> KT8 (used for the
                DMA-paced tile 0 so the first scores don't wait on later
                key chunks)."""
                sp = psc.tile([128, 2, QR], F32, tag="sc", name=f"kc_{t}_{jc}")
                for p in range(4):
                    nc.tensor.matmul(
                        sp[:, 0, :],
                        wkv[:, 2 * p : 2 * p + 2, t * 128 : (t + 1) * 128],
                        kv[:, 2 * p : 2 * p + 2, jc * 512 : (jc + 1) * 512],
                        start=(p == 0),
                        stop=(p == 3),
                        perf_mode=DR,
                    )
                nc.scalar.activation(
                    out=KT8[:, t, 4 * jc : 4 * jc + 4, :],
                    in_=sp[:, 0, :],
                    func=AF.Copy,
                    scale=1.0 / WSCALE,
                )

            def emit_vpair(dh, jp):
                """V proj d-half dh, key blocks jb=2jp,2jp+1 -> v8t[jp]."""
                sp = psc.tile([128, 2, QR], F32, tag="sc", name=f"vp_{dh}_{jp}")
                for i in range(2):
                    jb = 2 * jp + i
                    for p in range(4):
                        nc.tensor.matmul(
                            sp[:, i, :],
                            vv[:, jb, 2 * p : 2 * p + 2, :],
                            wvv[:, 2 * p : 2 * p + 2, dh * 512 : (dh + 1) * 512],
                            start=(p == 0),
                            stop=(p == 3),
                            perf_mode=DR,
                        )
                pvv = sp.rearrange("p s (h c) -> p s h c", h=8)
                nc.scalar.activation(
                    out=v8t[jp][:, 0:2, dh * 8 : (dh + 1) * 8, 0:64],
                    in_=pvv,
                    func=AF.Copy,
                    scale=1.0 / WSCALE,
                )

            # late-needed constants (issued after the big streams)
            wo_t = inp.tile([128, 8 * E], F8E4, tag="wo")
            nc.sync.dma_start(out=wo_t, in_=wo8[:, :])
            wov = wo_t.rearrange("p (s c) -> p s c", s=8)
            res_t = [
                inp.tile([128, E], BF16, tag="res", name=f"res_{qb}", bufs=4)
                for qb in range(4)
            ]
            for qb in range(4):
                nc.sync.dma_start(
                    out=res_t[qb], in_=resid[qb * 128 : (qb + 1) * 128, :]
                )
            gamma_b = constp.tile([128, E], BF16)
            nc.sync.dma_start(out=gamma_b, in_=vec3[0, :].partition_broadcast(128))
            beta_b = constp.tile([128, E], BF16)
            nc.sync.dma_start(out=beta_b, in_=vec3[1, :].partition_broadcast(128))
            id_t = constp.tile([128, 128], BF16)
            nc.sync.dma_start(out=id_t, in_=ident[:, :])
            csb = constp.tile([128, 8], F8E4)
            nc.sync.dma_start(out=csb, in_=cs8[:, :])
            rs_t = constp.tile([128, 4], BF16)
            nc.sync.dma_start(out=rs_t, in_=rs4[:, :])

            # ctx8: [128 d, 8 d-tile slot, 512 q] fp8e4 (x8 scale)
            ctx8 = ctxp.tile([128, 8, QR], F8E4, tag="ctx8")

            # ---------------- attention heads ----------------
            def emit_head(h, fill=None, norm_hook=None):
                t, r0 = h // 2, 64 * (h % 2)
                eng = PATTERNS[h]
                pc = pcp.tile([128, QR], F32, tag="pc", name=f"pc_{h}")
                for pair in range(8):
                    et = expp.tile([128, 2, QR], F8E5, tag="exp", name=f"e_{h}_{pair}")
                    sp = psc.tile([128, 2, QR], F32, tag="sc", name=f"s_{h}_{pair}")
                    for s in range(2):
                        jt = 2 * pair + s
                        nc.tensor.matmul(
                            sp[:, s, :],
                            KT8[r0 : r0 + 64, t, jt, :]
                            .unsqueeze(1)
                            .broadcast_to([64, 2, 128]),
                            QT8[r0 : r0 + 64, t, :, :],
                            start=True,
                            stop=True,
                            perf_mode=DR,
                        )
                    if eng[pair] == "A":
                        nc.scalar.activation(
                            out=et[:], in_=sp[:], func=AF.Exp, scale=0.125
                        )
                    else:
                        nc.vector.tensor_scalar(
                            out=et[:].bitcast(I8),
                            in0=sp[:],
                            scalar1=EXPA,
                            scalar2=EXPB,
                            op0=OP.mult,
                            op1=OP.add,
                        )
                    if fill is not None:
                        fill(8 * h + pair)
                    nc.tensor.matmul(
                        pc[0:65, :],
                        v8t[pair][:, :, h, :],
                        et[:],
                        start=(pair == 0),
                        stop=(pair == 7),
                        perf_mode=DR,
                    )
                    if norm_hook is not None:
                        norm_hook(pair)
                return pc

            # normalize: ctx8[d, i] = CTXSCALE * ctx~[d, i] / denom[i].
            # Split across the NEXT head's pair stream: reciprocal +
            # broadcast after pair 4, stt after pair 7 — the Pool
            # broadcast latency hides behind two more exps, so DVE never
            # stalls on it.
            def emit_recip(h, pc):
                rec = recp.tile([1, QR], F32, tag="rec", name=f"rc_{h}")
                nc.vector.reciprocal(out=rec, in_=pc[64:65, :])
                rb = recp.tile([64, QR], F32, tag="rb", name=f"rb_{h}")
                nc.gpsimd.partition_broadcast(rb[:], rec[:], channels=64)
                return rb

            def emit_stt(h, pc, rb):
                t, r0 = h // 2, 64 * (h % 2)
                nc.vector.scalar_tensor_tensor(
                    out=ctx8[r0 : r0 + 64, t, :],
                    in0=pc[0:64, :],
                    scalar=CTXSCALE,
                    in1=rb[:],
                    op0=OP.mult,
                    op1=OP.mult,
                )

            # fill schedule: (deadline pair-slot, kind, a, b). Slot s =
            # 8*h + pair. Deadlines are set just after the gating DMA's
            # estimated arrival so a late fill never blocks the in-order
            # PE/ACT streams.
            # V(0,jp) lands inside head 0 paced with the v8 DMA stream;
            # K tile 0's last chunks (kc2/kc3 arrive after v8) go in as
            # fills so they never block the early scores on the in-order
            # PE queue. Everything else is deadline-paced as before.
            fills = []
            for jp in range(8):
                fills.append((jp, "V", 0, jp))
            fills.append((3, "KC", 0, 2))
            fills.append((4, "KC", 0, 3))
            fills.append((8, "K", 1, 0))
            fills.append((10, "K", 1, 1))
            for jp in range(8):
                fills.append((16 + 5 * jp, "V", 1, jp))
            for t in range(2, 8):
                fills.append((16 * t - 12, "K", t, 0))
                fills.append((16 * t - 6, "K", t, 1))
            for tp in range(1, 4):
                fills.append((32 * tp - 8, "Q", tp, 0))
            fills.sort(key=lambda it: it[0])
            fill_state = {"i": 0}

            def emit_unit(kind, a, b2):
                if kind == "K":
                    emit_khalf(a, b2)
                elif kind == "KC":
                    emit_kchunk(a, b2)
                elif kind == "V":
                    emit_vpair(a, b2)
                else:
                    emit_qpair(a)

            def fill_one(s):
                while fill_state["i"] < len(fills) and fills[fill_state["i"]][0] <= s:
                    _, kind, a, b2 = fills[fill_state["i"]]
                    fill_state["i"] += 1
                    emit_unit(kind, a, b2)

            # prologue: Q tiles 0/1 and the first two K chunks of tile 0
            # (all the data the first scores pairs need)
            emit_qpair(0)
            emit_kchunk(0, 0)
            emit_kchunk(0, 1)

            prev = {"pc": None, "h": None, "rb": None}

            def norm_hook(pair):
                if prev["pc"] is None:
                    return
                if pair == 4:
                    prev["rb"] = emit_recip(prev["h"], prev["pc"])
                elif pair == 7:
                    emit_stt(prev["h"], prev["pc"], prev["rb"])
                    prev["pc"] = None

            for h in range(16):
                pc = emit_head(h, fill=fill_one, norm_hook=norm_hook)
                prev["pc"], prev["h"] = pc, h
            rb15 = emit_recip(15, prev["pc"])
            emit_stt(15, prev["pc"], rb15)
            # drain any leftover fills
            while fill_state["i"] < len(fills):
                _, kind, a, b2 = fills[fill_state["i"]]
                fill_state["i"] += 1
                emit_unit(kind, a, b2)

            # ------------- out projection + residual + LayerNorm -------------
            # Residual folds into the out-proj PSUM group via a 256*I
            # matmul, so PSUM holds 256*x; LN is scale-invariant once eps
            # is scaled by 256^2. Sum(x) comes from 9 tiny extra matmuls
            # (host-precomputed Wo column-sums + resid row-sums), Sum(x^2)
            # from an ACT Square pass with the free-axis accumulator —
            # bn_stats never touches DVE. rstd via DVE rsqrt bit-trick +
            # one Newton step; the (x-mu)*rstd affine is a single ACT
            # Identity(scale, bias) over both PSUM banks.
            MAGIC = float(0x5F3759DF)
            EPS256 = EPS * 65536.0
            po2s, sums, sx2s, ws = {}, {}, {}, {}

            def emit_po(qb):
                po2 = psc.tile([128, 2, QR], F32, tag="sc", name=f"po_{qb}")
                po2s[qb] = po2
                for eh in range(2):
                    po = po2[:, eh, :]
                    for p in range(4):
                        nc.tensor.matmul(
                            po[:],
                            ctx8[:, 2 * p : 2 * p + 2, qb * 128 : (qb + 1) * 128],
                            wov[:, 2 * p : 2 * p + 2, eh * 512 : (eh + 1) * 512],
                            start=(p == 0),
                            stop=False,
                            perf_mode=DR,
                        )
                    nc.tensor.matmul(
                        po,
                        id_t[:],
                        res_t[qb][:, eh * 512 : (eh + 1) * 512],
                        start=False,
                        stop=True,
                    )
                # 256*Sum(x) into one PSUM column
                sm = pcp.tile([128, QR], F32, tag="pc", name=f"sm_{qb}")
                sums[qb] = sm
                for sl in range(8):
                    nc.tensor.matmul(
                        sm[:, 0:1],
                        ctx8[:, sl, qb * 128 : (qb + 1) * 128],
                        csb[:, sl : sl + 1],
                        start=(sl == 0),
                        stop=False,
                    )
                nc.tensor.matmul(
                    sm[:, 0:1],
                    id_t[:],
                    rs_t[:, qb : qb + 1],
                    start=False,
                    stop=True,
                )

            def emit_square(qb):
                sq = lnp.tile([128, E], BF16, tag="sq", name=f"sq_{qb}")
                sx2 = lnp.tile([128, 1], F32, tag="sx2", name=f"sx2_{qb}", bufs=4)
                sx2s[qb] = sx2
                nc.scalar.activation(
                    out=sq[:].rearrange("p (s c) -> p s c", s=2),
                    in_=po2s[qb][:],
                    func=AF.Square,
                    accum_out=sx2[:, 0:1],
                )

            def emit_wchain(qb):
                # w cols: 0=mu(256x), 1=mu^2-eps256, 2=var+eps, 3=y0(bit),
                # 4=scratch, 5=rstd(256x), 6=-mu*rstd
                w = lnp.tile([128, 8], F32, tag="w", name=f"w_{qb}", bufs=4)
                ws[qb] = w
                nc.vector.tensor_scalar(
                    out=w[:, 0:1], in0=sums[qb][:, 0:1],
                    scalar1=1.0 / 1024.0, scalar2=None, op0=OP.mult,
                )
                nc.vector.tensor_scalar(
                    out=w[:, 1:2], in0=w[:, 0:1],
                    scalar1=w[:, 0:1], scalar2=-EPS256, op0=OP.mult, op1=OP.add,
                )
                nc.vector.scalar_tensor_tensor(
                    out=w[:, 2:3], in0=sx2s[qb][:, 0:1],
                    scalar=1.0 / 1024.0, in1=w[:, 1:2],
                    op0=OP.mult, op1=OP.subtract,
                )
                nc.vector.tensor_scalar(
                    out=w[:, 3:4].bitcast(mybir.dt.int32),
                    in0=w[:, 2:3].bitcast(mybir.dt.int32),
                    scalar1=-0.5, scalar2=MAGIC, op0=OP.mult, op1=OP.add,
                )
                nc.vector.tensor_tensor(
                    out=w[:, 4:5], in0=w[:, 3:4], in1=w[:, 3:4], op=OP.mult
                )
                nc.vector.tensor_tensor(
                    out=w[:, 4:5], in0=w[:, 4:5], in1=w[:, 2:3], op=OP.mult
                )
                nc.vector.tensor_scalar(
                    out=w[:, 4:5], in0=w[:, 4:5],
                    scalar1=-0.5, scalar2=1.5, op0=OP.mult, op1=OP.add,
                )
                nc.vector.tensor_tensor(
                    out=w[:, 5:6], in0=w[:, 3:4], in1=w[:, 4:5], op=OP.mult
                )
                nc.vector.tensor_scalar(
                    out=w[:, 6:7], in0=w[:, 0:1],
                    scalar1=w[:, 5:6], scalar2=-1.0, op0=OP.mult, op1=OP.mult,
                )

            def emit_ln_out(qb):
                w = ws[qb]
                a = lnp.tile([128, E], BF16, tag="a", name=f"a_{qb}")
                nc.scalar.activation(
                    out=a[:],
                    in_=po2s[qb][:],
                    func=AF.Identity,
                    bias=w[:, 6:7],
                    scale=w[:, 5:6],
                )
                y = lnp.tile([128, E], BF16, tag="y", name=f"y_{qb}")
                for eh in range(2):
                    sl = slice(eh * 512, (eh + 1) * 512)
                    nc.vector.tensor_tensor(
                        out=y[:, sl], in0=a[:, sl], in1=gamma_b[:, sl], op=OP.mult
                    )
                    nc.vector.tensor_tensor(
                        out=y[:, sl], in0=y[:, sl], in1=beta_b[:, sl], op=OP.add
                    )
                    nc.sync.dma_start(
                        out=out[qb * 128 : (qb + 1) * 128, sl], in_=y[:, sl]
                    )

            # software-pipelined emission: Square(qb+1) lands on ACT before
            # affine(qb) so ACT never waits on the DVE w-chain.
            emit_po(0)
            emit_square(0)
            emit_po(1)
            emit_square(1)
            emit_wchain(0)
            emit_po(2)
            emit_square(2)
            emit_ln_out(0)
            emit_wchain(1)
            emit_po(3)
            emit_square(3)
            emit_ln_out(1)
            emit_wchain(2)
            emit_ln_out(2)
            emit_wchain(3)
            emit_ln_out(3)

    nc.finalize()
    return nc


def _interleave_etiles(arr):
    """[E, N] -> [128, 8*N] with e = slot*128 + partition pairing layout."""
    Edim, N = arr.shape
    return np.ascontiguousarray(
        arr.reshape(8, 128, N).transpose(1, 0, 2).reshape(128, 8 * N)
    )


def build_in_maps(inputs):
    q = np.asarray(inputs["query"], dtype=np.float32)
    k = np.asarray(inputs["key"], dtype=np.float32)
    v = np.asarray(inputs["value"], dtype=np.float32)
    Wq = np.asarray(inputs["Wq"], dtype=np.float32)
    bq = np.asarray(inputs["bq"], dtype=np.float32)
    Wk = np.asarray(inputs["Wk"], dtype=np.float32)
    Wv = np.asarray(inputs["Wv"], dtype=np.float32)
    bv = np.asarray(inputs["bv"], dtype=np.float32)
    Wo = np.asarray(inputs["Wo"], dtype=np.float32)
    bo = np.asarray(inputs["bo"], dtype=np.float32)
    gamma = np.asarray(inputs["gamma"], dtype=np.float32)
    beta = np.asarray(inputs["beta"], dtype=np.float32)

    e4 = ml_dtypes.float8_e4m3
    # weights: pre-scaled x32, e = slot*128 + partition layout
    wq8 = _interleave_etiles(Wq.T * WSCALE).astype(e4)
    wk8 = _interleave_etiles(Wk.T * WSCALE).astype(e4)
    wv8 = _interleave_etiles(Wv.T * WSCALE).astype(e4)
    wo8 = _interleave_etiles(Wo.T * WSCALE).astype(e4)

    # per-batch activations
    k8 = [_interleave_etiles(np.ascontiguousarray(k[b].T)).astype(e4) for b in range(B)]
    v8 = []
    for b in range(B):
        t = _interleave_etiles(np.ascontiguousarray(v[b].T))  # [128, 8*2048]
        t = (
            t.reshape(128, 8, 16, 128)
            .transpose(0, 2, 1, 3)
            .reshape(128, 16 * 8 * 128)
        )
        v8.append(np.ascontiguousarray(t).astype(e4))

    # bv folded into a host-side bias vector: out includes +bv @ Wo.T + bo.
    bo_eff = (bv @ Wo.T + bo).astype(np.float32)
    # column sums of the (scaled, transposed) out-proj weights, used by the
    # device to compute Sum_e(out) with tiny matmuls (e = slot*128 + p)
    cs_vec = (Wo.sum(axis=0) * WSCALE).astype(np.float32)
    cs8 = np.ascontiguousarray(cs_vec.reshape(8, 128).T).astype(e4)

    in_maps = []
    for c in range(N_CORES):
        b, g = divmod(c, 2)
        rows = slice(QR * g, QR * g + QR)
        q8 = _interleave_etiles(np.ascontiguousarray(q[b, rows, :].T)).astype(e4)
        resid_bf = np.ascontiguousarray(q[b, rows, :] + bo_eff).astype(
            ml_dtypes.bfloat16
        )
        rs = resid_bf.astype(np.float32).sum(axis=1)  # [512]
        in_maps.append(
            {
                "q8": q8,
                "k8": k8[b],
                "v8": v8[b],
                "wq8": wq8,
                "wk8": wk8,
                "wv8": wv8,
                "wo8": wo8,
                "bq8": np.ascontiguousarray(bq.reshape(8, 128).T),
                "cs8": cs8,
                "rs4": np.ascontiguousarray(rs.reshape(4, 128).T).astype(
                    ml_dtypes.bfloat16
                ),
                "resid": resid_bf,
                "ident": (np.eye(128, dtype=np.float32) * (WSCALE * CTXSCALE)).astype(
                    ml_dtypes.bfloat16
                ),
                "vec3": np.ascontiguousarray(np.stack([gamma, beta])).astype(
                    ml_dtypes.bfloat16
                ),
            }
        )
    return in_maps


def kernel(**inputs):
    global _NC_CACHE, LAST_RESULTS
    if _NC_CACHE is None:
        _NC_CACHE = _build_nc()
    nc = _NC_CACHE

    in_maps = build_in_maps(inputs)

    res = run_bass_kernel_spmd(nc, in_maps, list(range(N_CORES)), trace=TRACE)
    LAST_RESULTS = res

    outp = np.empty((B, SQ, E), dtype=np.float32)
    for c in range(N_CORES):
        b, g = divmod(c, 2)
        outp[b, QR * g : QR * g + QR, :] = res.results[c]["out"].astype(np.float32)
    return outp


# revision 25
# speedup vs baseline: 1.0047x; 1.0047x over previous
"""Multi-head cross-modal attention + residual + LayerNorm on 8 TRN2 cores.

Reference computation (per batch b):
  Q = query @ Wq.T + bq ; K = key @ Wk.T + bk ; V = value @ Wv.T + bv
  attn = softmax(Q K^T / sqrt(D)) per head
  out  = (attn V) @ Wo.T + bo
  y    = LayerNorm(out + query) * gamma + beta

Sharding: 2-D over (batch=4) x (query-half=2). Core c owns batch c//2 and
queries [512*(c%2), 512*(c%2)+512); it computes ALL heads for its query
rows (K/V projections are duplicated across the pair of cores sharing a
batch), so there is NO collective: each core emits its own 512 rows of
the final LayerNorm output.

Precision/PE strategy: EVERY matmul (projections, scores, attn@V,
out-proj) runs fp8 with MatmulPerfMode.DoubleRow at 0.5 cycles/row.
Scores use a zero-slot DR trick: the per-head contraction is only 64, so
the DR pair's second slot is nulled by keeping a zeroed second slot in
the Q operand (QT8[:, t, 1, :] == 0); the K operand's second slot then
reads the next key block (or a zeroed pad column for jt=15) and
contributes exactly 0. Weights are pre-scaled x32 on host so fp8 sees
~N(0,1); the rescale folds into the PSUM->SBUF drain instructions.

bk is dropped entirely: K = Wk k + bk shifts every score of a given
query by the same constant (Q.bk), and softmax over keys is invariant to
per-query shifts. bv is folded on host into the residual (+ bv @ Wo.T).

Softmax: no max-subtraction (|score/8| < 7, fp32/e5m2 exp cannot
overflow). exp is split ACT/DVE per pair: ACT runs hardware Exp -> e5m2;
DVE runs a Schraudolph bit trick (round(a*score + b) written as int8,
bit-identical to e5m2) whose systematic error cancels in the softmax
ratio. Per-head normalization: DVE reciprocal of the ones-column
denominator row, Pool partition_broadcast down 64 partitions, one DVE
scalar_tensor_tensor into the fp8 ctx tile (x8 fp8 scale).

Drains (PSUM->SBUF) are batched two PSUM banks per instruction where no
per-partition bias is needed (K and V projections), and all memsets run
on the otherwise-idle Pool engine. DMA is issued in need-order so the
head pipeline starts as soon as the first K chunk lands.
"""

import sys

if "/opt/trn_rl_repo" not in sys.path:
    sys.path.insert(0, "/opt/trn_rl_repo")

import ml_dtypes
import numpy as np

import concourse.bass as bass  # noqa: F401  (registers types)
import concourse.mybir as mybir
import concourse.tile as tile
from concourse import bacc
from concourse.bass_utils import run_bass_kernel_spmd

F32 = mybir.dt.float32
BF16 = mybir.dt.bfloat16
F8E4 = mybir.dt.float8e4
F8E5 = mybir.dt.float8e5
I8 = mybir.dt.int8
AF = mybir.ActivationFunctionType
OP = mybir.AluOpType
DR = mybir.MatmulPerfMode.DoubleRow

B, SQ, SK, E, H, D = 4, 1024, 2048, 1024, 16, 64
N_CORES = 8
QR = 512           # queries per core
EPS = 1e-5
WSCALE = 32.0      # host-side weight pre-scale for fp8
CTXSCALE = 8.0     # fp8 scale applied to normalized ctx

# Schraudolph exp -> e5m2 bits: bits = trunc(EXPA * raw_score + EXPB)
EXPA = 0.125 * 4.0 / float(np.log(2.0))
EXPB = 58.5

# exp engine split per (head, jt-pair): A=ACT hw exp, D=DVE bit-trick.
# 64 A / 64 D overall; heads 0-1 are DVE-heavy (ACT carries the prologue
# drains while DMA streams in), heads 12-15 are ACT-heavy (DVE freed for
# the epilogue reciprocal/stt/LN chains).
PATTERNS = [
    "DDDDDDDD",  # h0
    "DDADDADA",  # h1
    "ADADADAD",  # h2
    "ADADADAD",  # h3
    "ADADADAD",  # h4
    "ADADADAD",  # h5
    "ADADADAD",  # h6
    "ADADADAD",  # h7
    "ADADADAD",  # h8
    "ADADADAD",  # h9
    "ADADADAD",  # h10
    "ADADADAD",  # h11
    "ADADADAD",  # h12
    "ADADADAD",  # h13
    "AADAADAA",  # h14
    "AADAADAA",  # h15
]

# module-level knobs used by test.py (harness ignores them)
TRACE = False
LAST_RESULTS = None

_NC_CACHE = None


def _build_nc():
    nc = bacc.Bacc(None, target_bir_lowering=False)

    q8 = nc.dram_tensor("q8", [128, 8 * QR], F8E4, kind="ExternalInput")
    k8 = nc.dram_tensor("k8", [128, 8 * SK], F8E4, kind="ExternalInput")
    v8 = nc.dram_tensor("v8", [128, 16 * 8 * 128], F8E4, kind="ExternalInput")
    wq8 = nc.dram_tensor("wq8", [128, 8 * E], F8E4, kind="ExternalInput")
    wk8 = nc.dram_tensor("wk8", [128, 8 * E], F8E4, kind="ExternalInput")
    wv8 = nc.dram_tensor("wv8", [128, 8 * E], F8E4, kind="ExternalInput")
    wo8 = nc.dram_tensor("wo8", [128, 8 * E], F8E4, kind="ExternalInput")
    bq8 = nc.dram_tensor("bq8", [128, 8], F32, kind="ExternalInput")
    cs8 = nc.dram_tensor("cs8", [128, 8], F8E4, kind="ExternalInput")
    rs4 = nc.dram_tensor("rs4", [128, 4], BF16, kind="ExternalInput")
    resid = nc.dram_tensor("resid", [QR, E], BF16, kind="ExternalInput")
    vec3 = nc.dram_tensor("vec3", [2, E], BF16, kind="ExternalInput")
    ident = nc.dram_tensor("ident", [128, 128], BF16, kind="ExternalInput")
    out = nc.dram_tensor("out", [QR, E], BF16, kind="ExternalOutput")

    from contextlib import ExitStack

    with ExitStack() as ctx:
        tc = ctx.enter_context(tile.TileContext(nc))
        constp = ctx.enter_context(tc.tile_pool(name="consts", bufs=1))
        inp = ctx.enter_context(tc.tile_pool(name="inp", bufs=1))
        qt8p = ctx.enter_context(tc.tile_pool(name="qt8p", bufs=1))
        kt8p = ctx.enter_context(tc.tile_pool(name="kt8p", bufs=1))
        vsb = ctx.enter_context(tc.tile_pool(name="vsb", bufs=8))
        expp = ctx.enter_context(tc.tile_pool(name="expp", bufs=4))
        ctxp = ctx.enter_context(tc.tile_pool(name="ctxp", bufs=1))
        recp = ctx.enter_context(tc.tile_pool(name="recp", bufs=2))
        lnp = ctx.enter_context(tc.tile_pool(name="lnp", bufs=2))
        # one 6-bank pool serves scores pairs, projection groups (both
        # slots) and the out-proj epilogue; pc accumulators get 2 banks.
        psc = ctx.enter_context(tc.tile_pool(name="psc", bufs=3, space="PSUM"))
        pcp = ctx.enter_context(tc.tile_pool(name="pcp", bufs=2, space="PSUM"))
        if True:
            # ---------------- input DMAs (need-order) ----------------
            bq_sb = constp.tile([128, 8], F32)
            nc.sync.dma_start(out=bq_sb, in_=bq8[:, :])
            wq_t = inp.tile([128, 8 * E], F8E4, tag="wq")
            nc.sync.dma_start(out=wq_t, in_=wq8[:, :])
            q_t = inp.tile([128, 8 * QR], F8E4, tag="q")
            nc.sync.dma_start(out=q_t, in_=q8[:, :])
            wk_t = inp.tile([128, 8 * E], F8E4, tag="wk")
            nc.sync.dma_start(out=wk_t, in_=wk8[:, :])
            k_t = inp.tile([128, 8 * SK], F8E4, tag="k")
            kvd = k_t.rearrange("p (s c) -> p s c", s=8)
            k8d = k8[:, :].rearrange("p (s c) -> p s c", s=8)
            for jc in range(2):
                nc.sync.dma_start(
                    out=kvd[:, :, jc * 512 : (jc + 1) * 512],
                    in_=k8d[:, :, jc * 512 : (jc + 1) * 512],
                )
            wv_t = inp.tile([128, 8 * E], F8E4, tag="wv")
            nc.sync.dma_start(out=wv_t, in_=wv8[:, :])
            v_t = inp.tile([128, 16 * 8 * 128], F8E4, tag="v")
            vvd = v_t.rearrange("p (jb r) -> p jb r", jb=16)
            v8d = v8[:, :].rearrange("p (jb r) -> p jb r", jb=16)
            for jp in range(8):
                nc.sync.dma_start(
                    out=vvd[:, 2 * jp : 2 * jp + 2, :],
                    in_=v8d[:, 2 * jp : 2 * jp + 2, :],
                )
            for jc in range(2, 4):
                nc.sync.dma_start(
                    out=kvd[:, :, jc * 512 : (jc + 1) * 512],
                    in_=k8d[:, :, jc * 512 : (jc + 1) * 512],
                )

            # slot views: (partition, slot, col)
            qv = q_t.rearrange("p (s c) -> p s c", s=8)
            kv = k_t.rearrange("p (s c) -> p s c", s=8)
            vv = v_t.rearrange("p (jb s c) -> p jb s c", jb=16, s=8)
            wqv = wq_t.rearrange("p (s c) -> p s c", s=8)
            wkv = wk_t.rearrange("p (s c) -> p s c", s=8)
            wvv = wv_t.rearrange("p (s c) -> p s c", s=8)

            # ------------- fp8 operand tiles for the head loop -------------
            # QT8: [128 d, 8 t, 2 slot, 512 q]; slot 1 stays 0 so the DR
            # pair's second contraction tile contributes nothing.
            QT8 = qt8p.tile([128, 8, 2, QR], F8E4)
            nc.gpsimd.memset(QT8[:, :, 1, :], 0.0)
            # KT8: [128 d, 8 t, 16 jt, 128 k]. The scores lhsT presents
            # each jt block twice via a stride-0 axis (broadcast_to), so
            # the DR pair's second slot re-reads the same block and is
            # nulled by QT8's zero slot — no pad, no cross-chunk reads.
            KT8 = kt8p.tile([128, 8, 16, 128], F8E4)
            # V8 tiles: per jt-pair [128 j, 2 slot, 16 head, 65] fp8e4
            v8t = [
                vsb.tile([128, 2, H, 65], F8E4, tag="v8", name=f"V8_{pr}")
                for pr in range(8)
            ]
            for pr in range(8):
                nc.gpsimd.memset(v8t[pr][:, :, :, 64:65], 1.0)

            # ---------------- projection emitters ----------------
            def emit_qpair(tp):
                """Q proj for tiles t=2tp, 2tp+1 -> QT8 slot 0 (fp8)."""
                sp = psc.tile([128, 2, QR], F32, tag="sc", name=f"qp_{tp}")
                for i in range(2):
                    t = 2 * tp + i
                    for p in range(4):
                        nc.tensor.matmul(
                            sp[:, i, :],
                            wqv[:, 2 * p : 2 * p + 2, t * 128 : (t + 1) * 128],
                            qv[:, 2 * p : 2 * p + 2, :],
                            start=(p == 0),
                            stop=(p == 3),
                            perf_mode=DR,
                        )
                for i in range(2):
                    t = 2 * tp + i
                    nc.scalar.activation(
                        out=QT8[:, t, 0, :],
                        in_=sp[:, i, :],
                        func=AF.Identity,
                        bias=bq_sb[:, t : t + 1],
                        scale=1.0 / WSCALE,
                    )

            def emit_khalf(t, half):
                """K proj tile t, key chunks jc=2*half,2*half+1 -> KT8."""
                sp = psc.tile([128, 2, QR], F32, tag="sc", name=f"kh_{t}_{half}")
                for i in range(2):
                    jc = 2 * half + i
                    for p in range(4):
                        nc.tensor.matmul(
                            sp[:, i, :],
                            wkv[:, 2 * p : 2 * p + 2, t * 128 : (t + 1) * 128],
                            kv[:, 2 * p : 2 * p + 2, jc * 512 : (jc + 1) * 512],
                            start=(p == 0),
                            stop=(p == 3),
                            perf_mode=DR,
                        )
                nc.scalar.activation(
                    out=KT8[:, t, 8 * half : 8 * half + 8, :],
                    in_=sp[:],
                    func=AF.Copy,
                    scale=1.0 / WSCALE,
                )

            def emit_kchunk(t, jc):
                """K proj tile t, single key chunk jc -> KT8 (used for the
                DMA-paced tile 0 so the first scores don't wait on later
                key chunks)."""
                sp = psc.tile([128, 2, QR], F32, tag="sc", name=f"kc_{t}_{jc}")
                for p in range(4):
                    nc.tensor.matmul(
                        sp[:, 0, :],
                        wkv[:, 2 * p : 2 * p + 2, t * 128 : (t + 1) * 128],
                        kv[:, 2 * p : 2 * p + 2, jc * 512 : (jc + 1) * 512],
                        start=(p == 0),
                        stop=(p == 3),
                        perf_mode=DR,
                    )
                nc.scalar.activation(
                    out=KT8[:, t, 4 * jc : 4 * jc + 4, :],
                    in_=sp[:, 0, :],
                    func=AF.Copy,
                    scale=1.0 / WSCALE,
                )

            def emit_vpair(dh, jp):
                """V proj d-half dh, key blocks jb=2jp,2jp+1 -> v8t[jp]."""
                sp = psc.tile([128, 2, QR], F32, tag="sc", name=f"vp_{dh}_{jp}")
                for i in range(2):
                    jb = 2 * jp + i
                    for p in range(4):
                        nc.tensor.matmul(
                            sp[:, i, :],
                            vv[:, jb, 2 * p : 2 * p + 2, :],
                            wvv[:, 2 * p : 2 * p + 2, dh * 512 : (dh + 1) * 512],
                            start=(p == 0),
                            stop=(p == 3),
                            perf_mode=DR,
                        )
                pvv = sp.rearrange("p s (h c) -> p s h c", h=8)
                nc.scalar.activation(
                    out=v8t[jp][:, 0:2, dh * 8 : (dh + 1) * 8, 0:64],
                    in_=pvv,
                    func=AF.Copy,
                    scale=1.0 / WSCALE,
                )

            # late-needed constants (issued after the big streams)
            wo_t = inp.tile([128, 8 * E], F8E4, tag="wo")
            nc.sync.dma_start(out=wo_t, in_=wo8[:, :])
            wov = wo_t.rearrange("p (s c) -> p s c", s=8)
            res_t = [
                inp.tile([128, E], BF16, tag="res", name=f"res_{qb}", bufs=4)
                for qb in range(4)
            ]
            for qb in range(4):
                nc.sync.dma_start(
                    out=res_t[qb], in_=resid[qb * 128 : (qb + 1) * 128, :]
                )
            gamma_b = constp.tile([128, E], BF16)
            nc.sync.dma_start(out=gamma_b, in_=vec3[0, :].partition_broadcast(128))
            beta_b = constp.tile([128, E], BF16)
            nc.sync.dma_start(out=beta_b, in_=vec3[1, :].partition_broadcast(128))
            id_t = constp.tile([128, 128], BF16)
            nc.sync.dma_start(out=id_t, in_=ident[:, :])
            csb = constp.tile([128, 8], F8E4)
            nc.sync.dma_start(out=csb, in_=cs8[:, :])
            rs_t = constp.tile([128, 4], BF16)
            nc.sync.dma_start(out=rs_t, in_=rs4[:, :])

            # ctx8: [128 d, 8 d-tile slot, 512 q] fp8e4 (x8 scale)
            ctx8 = ctxp.tile([128, 8, QR], F8E4, tag="ctx8")

            # ---------------- attention heads ----------------
            def emit_head(h, fill=None, norm_hook=None):
                t, r0 = h // 2, 64 * (h % 2)
                eng = PATTERNS[h]
                pc = pcp.tile([128, QR], F32, tag="pc", name=f"pc_{h}")
                for pair in range(8):
                    et = expp.tile([128, 2, QR], F8E5, tag="exp", name=f"e_{h}_{pair}")
                    sp = psc.tile([128, 2, QR], F32, tag="sc", name=f"s_{h}_{pair}")
                    for s in range(2):
                        jt = 2 * pair + s
                        nc.tensor.matmul(
                            sp[:, s, :],
                            KT8[r0 : r0 + 64, t, jt, :]
                            .unsqueeze(1)
                            .broadcast_to([64, 2, 128]),
                            QT8[r0 : r0 + 64, t, :, :],
                            start=True,
                            stop=True,
                            perf_mode=DR,
                        )
                    if eng[pair] == "A":
                        nc.scalar.activation(
                            out=et[:], in_=sp[:], func=AF.Exp, scale=0.125
                        )
                    else:
                        nc.vector.tensor_scalar(
                            out=et[:].bitcast(I8),
                            in0=sp[:],
                            scalar1=EXPA,
                            scalar2=EXPB,
                            op0=OP.mult,
                            op1=OP.add,
                        )
                    if fill is not None:
                        fill(8 * h + pair)
                    nc.tensor.matmul(
                        pc[0:65, :],
                        v8t[pair][:, :, h, :],
                        et[:],
                        start=(pair == 0),
                        stop=(pair == 7),
                        perf_mode=DR,
                    )
                    if norm_hook is not None:
                        norm_hook(pair)
                return pc

            # normalize: ctx8[d, i] = CTXSCALE * ctx~[d, i] / denom[i].
            # Split across the NEXT head's pair stream: reciprocal +
            # broadcast after pair 4, stt after pair 7 — the Pool
            # broadcast latency hides behind two more exps, so DVE never
            # stalls on it.
            def emit_recip(h, pc):
                rec = recp.tile([1, QR], F32, tag="rec", name=f"rc_{h}")
                nc.vector.reciprocal(out=rec, in_=pc[64:65, :])
                rb = recp.tile([64, QR], F32, tag="rb", name=f"rb_{h}")
                nc.gpsimd.partition_broadcast(rb[:], rec[:], channels=64)
                return rb

            def emit_stt(h, pc, rb):
                t, r0 = h // 2, 64 * (h % 2)
                nc.vector.scalar_tensor_tensor(
                    out=ctx8[r0 : r0 + 64, t, :],
                    in0=pc[0:64, :],
                    scalar=CTXSCALE,
                    in1=rb[:],
                    op0=OP.mult,
                    op1=OP.mult,
                )

            # fill schedule: (deadline pair-slot, kind, a, b). Slot s =
            # 8*h + pair. Deadlines are set just after the gating DMA's
            # estimated arrival so a late fill never blocks the in-order
            # PE/ACT streams.
            # V(0,jp) lands inside head 0 paced with the v8 DMA stream;
            # K tile 0's last chunks (kc2/kc3 arrive after v8) go in as
            # fills so they never block the early scores on the in-order
            # PE queue. Everything else is deadline-paced as before.
            fills = []
            for jp in range(8):
                fills.append((jp, "V", 0, jp))
            fills.append((3, "KC", 0, 2))
            fills.append((4, "KC", 0, 3))
            fills.append((8, "K", 1, 0))
            fills.append((10, "K", 1, 1))
            for jp in range(8):
                fills.append((16 + 5 * jp, "V", 1, jp))
            for t in range(2, 8):
                fills.append((16 * t - 12, "K", t, 0))
                fills.append((16 * t - 6, "K", t, 1))
            for tp in range(1, 4):
                fills.append((32 * tp - 8, "Q", tp, 0))
            fills.sort(key=lambda it: it[0])
            fill_state = {"i": 0}

            def emit_unit(kind, a, b2):
                if kind == "K":
                    emit_khalf(a, b2)
                elif kind == "KC":
                    emit_kchunk(a, b2)
                elif kind == "V":
                    emit_vpair(a, b2)
                else:
                    emit_qpair(a)

            def fill_one(s):
                while fill_state["i"] < len(fills) and fills[fill_state["i"]][0] <= s:
                    _, kind, a, b2 = fills[fill_state["i"]]
                    fill_state["i"] += 1
                    emit_unit(kind, a, b2)

            # prologue: Q tiles 0/1 and the first two K chunks of tile 0
            # (all the data the first scores pairs need)
            emit_qpair(0)
            emit_kchunk(0, 0)
            emit_kchunk(0, 1)

            prev = {"pc": None, "h": None, "rb": None}

            def norm_hook(pair):
                if prev["pc"] is None:
                    return
                if pair == 4:
                    prev["rb"] = emit_recip(prev["h"], prev["pc"])
                elif pair == 7:
                    emit_stt(prev["h"], prev["pc"], prev["rb"])
                    prev["pc"] = None

            for h in range(16):
                pc = emit_head(h, fill=fill_one, norm_hook=norm_hook)
                prev["pc"], prev["h"] = pc, h
            rb15 = emit_recip(15, prev["pc"])
            emit_stt(15, prev["pc"], rb15)
            # drain any leftover fills
            while fill_state["i"] < len(fills):
                _, kind, a, b2 = fills[fill_state["i"]]
                fill_state["i"] += 1
                emit_unit(kind, a, b2)

            # ------------- out projection + residual + LayerNorm -------------
            # Residual folds into the out-proj PSUM group via a 256*I
            # matmul, so PSUM holds 256*x; LN is scale-invariant once eps
            # is scaled by 256^2. Sum(x) comes from 9 tiny extra matmuls
            # (host-precomputed Wo column-sums + resid row-sums), Sum(x^2)
            # from an ACT Square pass with the free-axis accumulator —
            # bn_stats never touches DVE. rstd via DVE rsqrt bit-trick +
            # one Newton step; the (x-mu)*rstd affine is a single ACT
            # Identity(scale, bias) over both PSUM banks.
            MAGIC = float(0x5F3759DF)
            EPS256 = EPS * 65536.0
            po2s, sums, sx2s, ws = {}, {}, {}, {}

            def emit_po(qb):
                po2 = psc.tile([128, 2, QR], F32, tag="sc", name=f"po_{qb}")
                po2s[qb] = po2
                for eh in range(2):
                    po = po2[:, eh, :]
                    for p in range(4):
                        nc.tensor.matmul(
                            po[:],
                            ctx8[:, 2 * p : 2 * p + 2, qb * 128 : (qb + 1) * 128],
                            wov[:, 2 * p : 2 * p + 2, eh * 512 : (eh + 1) * 512],
                            start=(p == 0),
                            stop=False,
                            perf_mode=DR,
                        )
                    nc.tensor.matmul(
                        po,
                        id_t[:],
                        res_t[qb][:, eh * 512 : (eh + 1) * 512],
                        start=False,
                        stop=True,
                    )
                # 256*Sum(x) into one PSUM column
                sm = pcp.tile([128, QR], F32, tag="pc", name=f"sm_{qb}")
                sums[qb] = sm
                for sl in range(8):
                    nc.tensor.matmul(
                        sm[:, 0:1],
                        ctx8[:, sl, qb * 128 : (qb + 1) * 128],
                        csb[:, sl : sl + 1],
                        start=(sl == 0),
                        stop=False,
                    )
                nc.tensor.matmul(
                    sm[:, 0:1],
                    id_t[:],
                    rs_t[:, qb : qb + 1],
                    start=False,
                    stop=True,
                )

            def emit_square(qb):
                sq = lnp.tile([128, E], BF16, tag="sq", name=f"sq_{qb}")
                sx2 = lnp.tile([128, 1], F32, tag="sx2", name=f"sx2_{qb}", bufs=4)
                sx2s[qb] = sx2
                nc.scalar.activation(
                    out=sq[:].rearrange("p (s c) -> p s c", s=2),
                    in_=po2s[qb][:],
                    func=AF.Square,
                    accum_out=sx2[:, 0:1],
                )

            def emit_wchain(qb):
                # w cols: 0=mu(256x), 1=mu^2-eps256, 2=var+eps, 3=y0(bit),
                # 4=scratch, 5=rstd(256x), 6=-mu*rstd
                w = lnp.tile([128, 8], F32, tag="w", name=f"w_{qb}", bufs=4)
                ws[qb] = w
                nc.vector.tensor_scalar(
                    out=w[:, 0:1], in0=sums[qb][:, 0:1],
                    scalar1=1.0 / 1024.0, scalar2=None, op0=OP.mult,
                )
                nc.vector.tensor_scalar(
                    out=w[:, 1:2], in0=w[:, 0:1],
                    scalar1=w[:, 0:1], scalar2=-EPS256, op0=OP.mult, op1=OP.add,
                )
                nc.vector.scalar_tensor_tensor(
                    out=w[:, 2:3], in0=sx2s[qb][:, 0:1],
                    scalar=1.0 / 1024.0, in1=w[:, 1:2],
                    op0=OP.mult, op1=OP.subtract,
                )
                nc.vector.tensor_scalar(
                    out=w[:, 3:4].bitcast(mybir.dt.int32),
                    in0=w[:, 2:3].bitcast(mybir.dt.int32),
                    scalar1=-0.5, scalar2=MAGIC, op0=OP.mult, op1=OP.add,
                )
                nc.vector.tensor_tensor(
                    out=w[:, 4:5], in0=w[:, 3:4], in1=w[:, 3:4], op=OP.mult
                )
                nc.vector.tensor_tensor(
                    out=w[:, 4:5], in0=w[:, 4:5], in1=w[:, 2:3], op=OP.mult
                )
                nc.vector.tensor_scalar(
                    out=w[:, 4:5], in0=w[:, 4:5],
                    scalar1=-0.5, scalar2=1.5, op0=OP.mult, op1=OP.add,
                )
                nc.vector.tensor_tensor(
                    out=w[:, 5:6], in0=w[:, 3:4], in1=w[:, 4:5], op=OP.mult
                )
                nc.vector.tensor_scalar(
                    out=w[:, 6:7], in0=w[:, 0:1],
                    scalar1=w[:, 5:6], scalar2=-1.0, op0=OP.mult, op1=OP.mult,
                )

            def emit_ln_out(qb):
                w = ws[qb]
                a = lnp.tile([128, E], BF16, tag="a", name=f"a_{qb}")
                nc.scalar.activation(
                    out=a[:],
                    in_=po2s[qb][:],
                    func=AF.Identity,
                    bias=w[:, 6:7],
                    scale=w[:, 5:6],
                )
                y = lnp.tile([128, E], BF16, tag="y", name=f"y_{qb}")
                nc.vector.tensor_tensor(out=y, in0=a, in1=gamma_b, op=OP.mult)
                nc.vector.tensor_tensor(out=y, in0=y, in1=beta_b, op=OP.add)
                nc.sync.dma_start(
                    out=out[qb * 128 : (qb + 1) * 128, :], in_=y
                )

            # software-pipelined emission: Square(qb+1) lands on ACT before
            # affine(qb) so ACT never waits on the DVE w-chain.
            emit_po(0)
            emit_square(0)
            emit_po(1)
            emit_square(1)
            emit_wchain(0)
            emit_po(2)
            emit_square(2)
            emit_ln_out(0)
            emit_wchain(1)
            emit_po(3)
            emit_square(3)
            emit_ln_out(1)
            emit_wchain(2)
            emit_ln_out(2)
            emit_wchain(3)
            emit_ln_out(3)

    nc.finalize()
    return nc


def _interleave_etiles(arr):
    """[E, N] -> [128, 8*N] with e = slot*128 + partition pairing layout."""
    Edim, N = arr.shape
    return np.ascontiguousarray(
        arr.reshape(8, 128, N).transpose(1, 0, 2).reshape(128, 8 * N)
    )


def build_in_maps(inputs):
    q = np.asarray(inputs["query"], dtype=np.float32)
    k = np.asarray(inputs["key"], dtype=np.float32)
    v = np.asarray(inputs["value"], dtype=np.float32)
    Wq = np.asarray(inputs["Wq"], dtype=np.float32)
    bq = np.asarray(inputs["bq"], dtype=np.float32)
    Wk = np.asarray(inputs["Wk"], dtype=np.float32)
    Wv = np.asarray(inputs["Wv"], dtype=np.float32)
    bv = np.asarray(inputs["bv"], dtype=np.float32)
    Wo = np.asarray(inputs["Wo"], dtype=np.float32)
    bo = np.asarray(inputs["bo"], dtype=np.float32)
    gamma = np.asarray(inputs["gamma"], dtype=np.float32)
    beta = np.asarray(inputs["beta"], dtype=np.float32)

    e4 = ml_dtypes.float8_e4m3
    # weights: pre-scaled x32, e = slot*128 + partition layout
    wq8 = _interleave_etiles(Wq.T * WSCALE).astype(e4)
    wk8 = _interleave_etiles(Wk.T * WSCALE).astype(e4)
    wv8 = _interleave_etiles(Wv.T * WSCALE).astype(e4)
    wo8 = _interleave_etiles(Wo.T * WSCALE).astype(e4)

    # per-batch activations
    k8 = [_interleave_etiles(np.ascontiguousarray(k[b].T)).astype(e4) for b in range(B)]
    v8 = []
    for b in range(B):
        t = _interleave_etiles(np.ascontiguousarray(v[b].T))  # [128, 8*2048]
        t = (
            t.reshape(128, 8, 16, 128)
            .transpose(0, 2, 1, 3)
            .reshape(128, 16 * 8 * 128)
        )
        v8.append(np.ascontiguousarray(t).astype(e4))

    # bv folded into a host-side bias vector: out includes +bv @ Wo.T + bo.
    bo_eff = (bv @ Wo.T + bo).astype(np.float32)
    # column sums of the (scaled, transposed) out-proj weights, used by the
    # device to compute Sum_e(out) with tiny matmuls (e = slot*128 + p)
    cs_vec = (Wo.sum(axis=0) * WSCALE).astype(np.float32)
    cs8 = np.ascontiguousarray(cs_vec.reshape(8, 128).T).astype(e4)

    in_maps = []
    for c in range(N_CORES):
        b, g = divmod(c, 2)
        rows = slice(QR * g, QR * g + QR)
        q8 = _interleave_etiles(np.ascontiguousarray(q[b, rows, :].T)).astype(e4)
        resid_bf = np.ascontiguousarray(q[b, rows, :] + bo_eff).astype(
            ml_dtypes.bfloat16
        )
        rs = resid_bf.astype(np.float32).sum(axis=1)  # [512]
        in_maps.append(
            {
                "q8": q8,
                "k8": k8[b],
                "v8": v8[b],
                "wq8": wq8,
                "wk8": wk8,
                "wv8": wv8,
                "wo8": wo8,
                "bq8": np.ascontiguousarray(bq.reshape(8, 128).T),
                "cs8": cs8,
                "rs4": np.ascontiguousarray(rs.reshape(4, 128).T).astype(
                    ml_dtypes.bfloat16
                ),
                "resid": resid_bf,
                "ident": (np.eye(128, dtype=np.float32) * (WSCALE * CTXSCALE)).astype(
                    ml_dtypes.bfloat16
                ),
                "vec3": np.ascontiguousarray(np.stack([gamma, beta])).astype(
                    ml_dtypes.bfloat16
                ),
            }
        )
    return in_maps


def kernel(**inputs):
    global _NC_CACHE, LAST_RESULTS
    if _NC_CACHE is None:
        _NC_CACHE = _build_nc()
    nc = _NC_CACHE

    in_maps = build_in_maps(inputs)

    res = run_bass_kernel_spmd(nc, in_maps, list(range(N_CORES)), trace=TRACE)
    LAST_RESULTS = res

    outp = np.empty((B, SQ, E), dtype=np.float32)
    for c in range(N_CORES):
        b, g = divmod(c, 2)
        outp[b, QR * g : QR * g + QR, :] = res.results[c]["out"].astype(np.float32)
    return outp


# revision 29
# speedup vs baseline: 1.0501x; 1.0452x over previous
"""Multi-head cross-modal attention + residual + LayerNorm on 8 TRN2 cores.

Reference computation (per batch b):
  Q = query @ Wq.T + bq ; K = key @ Wk.T + bk ; V = value @ Wv.T + bv
  attn = softmax(Q K^T / sqrt(D)) per head
  out  = (attn V) @ Wo.T + bo
  y    = LayerNorm(out + query) * gamma + beta

Sharding: 2-D over (batch=4) x (query-half=2). Core c owns batch c//2 and
queries [512*(c%2), 512*(c%2)+512); it computes ALL heads for its query
rows (K/V projections are duplicated across the pair of cores sharing a
batch), so there is NO collective: each core emits its own 512 rows of
the final LayerNorm output.

Precision/PE strategy: EVERY matmul (projections, scores, attn@V,
out-proj) runs fp8 with MatmulPerfMode.DoubleRow at 0.5 cycles/row.
Scores use a zero-slot DR trick: the per-head contraction is only 64, so
the DR pair's second slot is nulled by keeping a zeroed second slot in
the Q operand (QT8[:, t, 1, :] == 0); the K operand's second slot then
reads the next key block (or a zeroed pad column for jt=15) and
contributes exactly 0. Weights are pre-scaled x32 on host so fp8 sees
~N(0,1); the rescale folds into the PSUM->SBUF drain instructions.

bk is dropped entirely: K = Wk k + bk shifts every score of a given
query by the same constant (Q.bk), and softmax over keys is invariant to
per-query shifts. bv is folded on host into the residual (+ bv @ Wo.T).

Softmax: no max-subtraction (|score/8| < 7, fp32/e5m2 exp cannot
overflow). exp is split ACT/DVE per pair: ACT runs hardware Exp -> e5m2;
DVE runs a Schraudolph bit trick (round(a*score + b) written as int8,
bit-identical to e5m2) whose systematic error cancels in the softmax
ratio. Per-head normalization: DVE reciprocal of the ones-column
denominator row, Pool partition_broadcast down 64 partitions, one DVE
scalar_tensor_tensor into the fp8 ctx tile (x8 fp8 scale).

Drains (PSUM->SBUF) are batched two PSUM banks per instruction where no
per-partition bias is needed (K and V projections), and all memsets run
on the otherwise-idle Pool engine. DMA is issued in need-order so the
head pipeline starts as soon as the first K chunk lands.
"""

import sys

if "/opt/trn_rl_repo" not in sys.path:
    sys.path.insert(0, "/opt/trn_rl_repo")

import ml_dtypes
import numpy as np

import concourse.bass as bass  # noqa: F401  (registers types)
import concourse.mybir as mybir
import concourse.tile as tile
from concourse import bacc
from concourse.bass_utils import run_bass_kernel_spmd

F32 = mybir.dt.float32
BF16 = mybir.dt.bfloat16
F8E4 = mybir.dt.float8e4
F8E5 = mybir.dt.float8e5
I8 = mybir.dt.int8
AF = mybir.ActivationFunctionType
OP = mybir.AluOpType
DR = mybir.MatmulPerfMode.DoubleRow

B, SQ, SK, E, H, D = 4, 1024, 2048, 1024, 16, 64
N_CORES = 8
QR = 512           # queries per core
EPS = 1e-5
WSCALE = 32.0      # host-side weight pre-scale for fp8
CTXSCALE = 8.0     # fp8 scale applied to normalized ctx

# Schraudolph exp -> e5m2 bits: bits = trunc(EXPA * raw_score + EXPB)
EXPA = 0.125 * 4.0 / float(np.log(2.0))
EXPB = 58.5

# exp engine split per (head, jt-pair): A=ACT hw exp, D=DVE bit-trick.
# 64 A / 64 D overall; heads 0-1 are DVE-heavy (ACT carries the prologue
# drains while DMA streams in), heads 12-15 are ACT-heavy (DVE freed for
# the epilogue reciprocal/stt/LN chains).
PATTERNS = [
    "DDDDDDDD",  # h0
    "DDADDADA",  # h1
    "ADADADAD",  # h2
    "ADADADAD",  # h3
    "ADADADAD",  # h4
    "ADADADAD",  # h5
    "ADADADAD",  # h6
    "ADADADAD",  # h7
    "ADADADAD",  # h8
    "ADADADAD",  # h9
    "ADADADAD",  # h10
    "ADADADAD",  # h11
    "ADADADAD",  # h12
    "ADADADAD",  # h13
    "AADAADAA",  # h14
    "AADAADAA",  # h15
]

# module-level knobs used by test.py (harness ignores them)
TRACE = False
LAST_RESULTS = None

_NC_CACHE = None


def _build_nc():
    nc = bacc.Bacc(None, target_bir_lowering=False)

    q8 = nc.dram_tensor("q8", [128, 8 * QR], F8E4, kind="ExternalInput")
    k8 = nc.dram_tensor("k8", [128, 8 * SK], F8E4, kind="ExternalInput")
    v8 = nc.dram_tensor("v8", [128, 16 * 8 * 128], F8E4, kind="ExternalInput")
    wq8 = nc.dram_tensor("wq8", [128, 8 * E], F8E4, kind="ExternalInput")
    wk8 = nc.dram_tensor("wk8", [128, 8 * E], F8E4, kind="ExternalInput")
    wv8 = nc.dram_tensor("wv8", [128, 8 * E], F8E4, kind="ExternalInput")
    wo8 = nc.dram_tensor("wo8", [128, 8 * E], F8E4, kind="ExternalInput")
    bq8 = nc.dram_tensor("bq8", [128, 8], F32, kind="ExternalInput")
    cs8 = nc.dram_tensor("cs8", [128, 8], F8E4, kind="ExternalInput")
    rs4 = nc.dram_tensor("rs4", [128, 4], BF16, kind="ExternalInput")
    resid = nc.dram_tensor("resid", [QR, E], BF16, kind="ExternalInput")
    vec3 = nc.dram_tensor("vec3", [2, E], BF16, kind="ExternalInput")
    ident = nc.dram_tensor("ident", [128, 128], BF16, kind="ExternalInput")
    out = nc.dram_tensor("out", [QR, E], BF16, kind="ExternalOutput")

    from contextlib import ExitStack

    with ExitStack() as ctx:
        tc = ctx.enter_context(tile.TileContext(nc))
        constp = ctx.enter_context(tc.tile_pool(name="consts", bufs=1))
        inp = ctx.enter_context(tc.tile_pool(name="inp", bufs=1))
        qt8p = ctx.enter_context(tc.tile_pool(name="qt8p", bufs=1))
        kt8p = ctx.enter_context(tc.tile_pool(name="kt8p", bufs=1))
        vsb = ctx.enter_context(tc.tile_pool(name="vsb", bufs=8))
        expp = ctx.enter_context(tc.tile_pool(name="expp", bufs=16))
        ctxp = ctx.enter_context(tc.tile_pool(name="ctxp", bufs=1))
        recp = ctx.enter_context(tc.tile_pool(name="recp", bufs=2))
        lnp = ctx.enter_context(tc.tile_pool(name="lnp", bufs=2))
        # one 6-bank pool serves scores pairs, projection groups (both
        # slots) and the out-proj epilogue; pc accumulators get 2 banks.
        psc = ctx.enter_context(tc.tile_pool(name="psc", bufs=3, space="PSUM"))
        pcp = ctx.enter_context(tc.tile_pool(name="pcp", bufs=2, space="PSUM"))
        if True:
            # ---------------- input DMAs (need-order) ----------------
            # Tile-0 weight columns first: heads 0 AND 1 both live in
            # K/Q tile 0, so loading just those 128 columns (0.13 MB each)
            # lets the first scores fire at ~6.5us instead of ~11us.
            bq_sb = constp.tile([128, 8], F32)
            nc.sync.dma_start(out=bq_sb, in_=bq8[:, :])
            wq_t = inp.tile([128, 8 * E], F8E4, tag="wq")
            wqd = wq_t.rearrange("p (s c) -> p s c", s=8)
            wq8d = wq8[:, :].rearrange("p (s c) -> p s c", s=8)
            nc.sync.dma_start(out=wqd[:, :, 0:128], in_=wq8d[:, :, 0:128])
            q_t = inp.tile([128, 8 * QR], F8E4, tag="q")
            nc.sync.dma_start(out=q_t, in_=q8[:, :])
            wk_t = inp.tile([128, 8 * E], F8E4, tag="wk")
            wkd = wk_t.rearrange("p (s c) -> p s c", s=8)
            wk8d = wk8[:, :].rearrange("p (s c) -> p s c", s=8)
            nc.sync.dma_start(out=wkd[:, :, 0:128], in_=wk8d[:, :, 0:128])
            k_t = inp.tile([128, 8 * SK], F8E4, tag="k")
            kvd = k_t.rearrange("p (s c) -> p s c", s=8)
            k8d = k8[:, :].rearrange("p (s c) -> p s c", s=8)
            for jc in range(2):
                nc.sync.dma_start(
                    out=kvd[:, :, jc * 512 : (jc + 1) * 512],
                    in_=k8d[:, :, jc * 512 : (jc + 1) * 512],
                )
            nc.sync.dma_start(out=wkd[:, :, 128:1024], in_=wk8d[:, :, 128:1024])
            for jc in range(2, 4):
                nc.sync.dma_start(
                    out=kvd[:, :, jc * 512 : (jc + 1) * 512],
                    in_=k8d[:, :, jc * 512 : (jc + 1) * 512],
                )
            nc.sync.dma_start(out=wqd[:, :, 128:1024], in_=wq8d[:, :, 128:1024])
            wv_t = inp.tile([128, 8 * E], F8E4, tag="wv")
            nc.sync.dma_start(out=wv_t, in_=wv8[:, :])
            v_t = inp.tile([128, 16 * 8 * 128], F8E4, tag="v")
            vvd = v_t.rearrange("p (jb r) -> p jb r", jb=16)
            v8d = v8[:, :].rearrange("p (jb r) -> p jb r", jb=16)
            for jp in range(8):
                nc.sync.dma_start(
                    out=vvd[:, 2 * jp : 2 * jp + 2, :],
                    in_=v8d[:, 2 * jp : 2 * jp + 2, :],
                )

            # slot views: (partition, slot, col)
            qv = q_t.rearrange("p (s c) -> p s c", s=8)
            kv = k_t.rearrange("p (s c) -> p s c", s=8)
            vv = v_t.rearrange("p (jb s c) -> p jb s c", jb=16, s=8)
            wqv = wq_t.rearrange("p (s c) -> p s c", s=8)
            wkv = wk_t.rearrange("p (s c) -> p s c", s=8)
            wvv = wv_t.rearrange("p (s c) -> p s c", s=8)

            # ------------- fp8 operand tiles for the head loop -------------
            # QT8: [128 d, 8 t, 2 slot, 512 q]; slot 1 stays 0 so the DR
            # pair's second contraction tile contributes nothing.
            QT8 = qt8p.tile([128, 8, 2, QR], F8E4)
            nc.gpsimd.memset(QT8[:, :, 1, :], 0.0)
            # KT8: [128 d, 8 t, 16 jt, 128 k]. The scores lhsT presents
            # each jt block twice via a stride-0 axis (broadcast_to), so
            # the DR pair's second slot re-reads the same block and is
            # nulled by QT8's zero slot — no pad, no cross-chunk reads.
            KT8 = kt8p.tile([128, 8, 16, 128], F8E4)
            # V8 tiles: per jt-pair [128 j, 2 slot, 16 head, 65] fp8e4
            v8t = [
                vsb.tile([128, 2, H, 65], F8E4, tag="v8", name=f"V8_{pr}")
                for pr in range(8)
            ]
            for pr in range(8):
                nc.gpsimd.memset(v8t[pr][:, :, :, 64:65], 1.0)

            # ---------------- projection emitters ----------------
            def emit_qpair(tp):
                """Q proj for tiles t=2tp, 2tp+1 -> QT8 slot 0 (fp8)."""
                sp = psc.tile([128, 2, QR], F32, tag="sc", name=f"qp_{tp}")
                for i in range(2):
                    t = 2 * tp + i
                    for p in range(4):
                        nc.tensor.matmul(
                            sp[:, i, :],
                            wqv[:, 2 * p : 2 * p + 2, t * 128 : (t + 1) * 128],
                            qv[:, 2 * p : 2 * p + 2, :],
                            start=(p == 0),
                            stop=(p == 3),
                            perf_mode=DR,
                        )
                for i in range(2):
                    t = 2 * tp + i
                    nc.scalar.activation(
                        out=QT8[:, t, 0, :],
                        in_=sp[:, i, :],
                        func=AF.Identity,
                        bias=bq_sb[:, t : t + 1],
                        scale=1.0 / WSCALE,
                    )

            def emit_khalf(t, half):
                """K proj tile t, key chunks jc=2*half,2*half+1 -> KT8."""
                sp = psc.tile([128, 2, QR], F32, tag="sc", name=f"kh_{t}_{half}")
                for i in range(2):
                    jc = 2 * half + i
                    for p in range(4):
                        nc.tensor.matmul(
                            sp[:, i, :],
                            wkv[:, 2 * p : 2 * p + 2, t * 128 : (t + 1) * 128],
                            kv[:, 2 * p : 2 * p + 2, jc * 512 : (jc + 1) * 512],
                            start=(p == 0),
                            stop=(p == 3),
                            perf_mode=DR,
                        )
                nc.scalar.activation(
                    out=KT8[:, t, 8 * half : 8 * half + 8, :],
                    in_=sp[:],
                    func=AF.Copy,
                    scale=1.0 / WSCALE,
                )

            def emit_qsingle(t):
                """Q proj for a single tile t -> QT8 slot 0 (fp8)."""
                sp = psc.tile([128, 2, QR], F32, tag="sc", name=f"qs_{t}")
                for p in range(4):
                    nc.tensor.matmul(
                        sp[:, 0, :],
                        wqv[:, 2 * p : 2 * p + 2, t * 128 : (t + 1) * 128],
                        qv[:, 2 * p : 2 * p + 2, :],
                        start=(p == 0),
                        stop=(p == 3),
                        perf_mode=DR,
                    )
                nc.scalar.activation(
                    out=QT8[:, t, 0, :],
                    in_=sp[:, 0, :],
                    func=AF.Identity,
                    bias=bq_sb[:, t : t + 1],
                    scale=1.0 / WSCALE,
                )

            def emit_kchunk(t, jc):
                """K proj tile t, single key chunk jc -> KT8 (used for the
                DMA-paced tile 0 so the first scores don't wait on later
                key chunks)."""
                sp = psc.tile([128, 2, QR], F32, tag="sc", name=f"kc_{t}_{jc}")
                for p in range(4):
                    nc.tensor.matmul(
                        sp[:, 0, :],
                        wkv[:, 2 * p : 2 * p + 2, t * 128 : (t + 1) * 128],
                        kv[:, 2 * p : 2 * p + 2, jc * 512 : (jc + 1) * 512],
                        start=(p == 0),
                        stop=(p == 3),
                        perf_mode=DR,
                    )
                nc.scalar.activation(
                    out=KT8[:, t, 4 * jc : 4 * jc + 4, :],
                    in_=sp[:, 0, :],
                    func=AF.Copy,
                    scale=1.0 / WSCALE,
                )

            def emit_vpair(dh, jp):
                """V proj d-half dh, key blocks jb=2jp,2jp+1 -> v8t[jp]."""
                sp = psc.tile([128, 2, QR], F32, tag="sc", name=f"vp_{dh}_{jp}")
                for i in range(2):
                    jb = 2 * jp + i
                    for p in range(4):
                        nc.tensor.matmul(
                            sp[:, i, :],
                            vv[:, jb, 2 * p : 2 * p + 2, :],
                            wvv[:, 2 * p : 2 * p + 2, dh * 512 : (dh + 1) * 512],
                            start=(p == 0),
                            stop=(p == 3),
                            perf_mode=DR,
                        )
                pvv = sp.rearrange("p s (h c) -> p s h c", h=8)
                nc.scalar.activation(
                    out=v8t[jp][:, 0:2, dh * 8 : (dh + 1) * 8, 0:64],
                    in_=pvv,
                    func=AF.Copy,
                    scale=1.0 / WSCALE,
                )

            # late-needed constants (issued after the big streams)
            wo_t = inp.tile([128, 8 * E], F8E4, tag="wo")
            nc.sync.dma_start(out=wo_t, in_=wo8[:, :])
            wov = wo_t.rearrange("p (s c) -> p s c", s=8)
            res_t = [
                inp.tile([128, E], BF16, tag="res", name=f"res_{qb}", bufs=4)
                for qb in range(4)
            ]
            for qb in range(4):
                nc.sync.dma_start(
                    out=res_t[qb], in_=resid[qb * 128 : (qb + 1) * 128, :]
                )
            gamma_b = constp.tile([128, E], BF16)
            nc.sync.dma_start(out=gamma_b, in_=vec3[0, :].partition_broadcast(128))
            beta_b = constp.tile([128, E], BF16)
            nc.sync.dma_start(out=beta_b, in_=vec3[1, :].partition_broadcast(128))
            id_t = constp.tile([128, 128], BF16)
            nc.sync.dma_start(out=id_t, in_=ident[:, :])
            csb = constp.tile([128, 8], F8E4)
            nc.sync.dma_start(out=csb, in_=cs8[:, :])
            rs_t = constp.tile([128, 4], BF16)
            nc.sync.dma_start(out=rs_t, in_=rs4[:, :])

            # ctx8: [128 d, 8 d-tile slot, 512 q] fp8e4 (x8 scale)
            ctx8 = ctxp.tile([128, 8, QR], F8E4, tag="ctx8")

            # ---------------- attention heads ----------------
            def emit_head(h, fill=None, norm_hook=None):
                t, r0 = h // 2, 64 * (h % 2)
                eng = PATTERNS[h]
                pc = pcp.tile([128, QR], F32, tag="pc", name=f"pc_{h}")
                for pair in range(8):
                    et = expp.tile([128, 2, QR], F8E5, tag="exp", name=f"e_{h}_{pair}")
                    sp = psc.tile([128, 2, QR], F32, tag="sc", name=f"s_{h}_{pair}")
                    for s in range(2):
                        jt = 2 * pair + s
                        nc.tensor.matmul(
                            sp[:, s, :],
                            KT8[r0 : r0 + 64, t, jt, :]
                            .unsqueeze(1)
                            .broadcast_to([64, 2, 128]),
                            QT8[r0 : r0 + 64, t, :, :],
                            start=True,
                            stop=True,
                            perf_mode=DR,
                        )
                    if eng[pair] == "A":
                        nc.scalar.activation(
                            out=et[:], in_=sp[:], func=AF.Exp, scale=0.125
                        )
                    else:
                        nc.vector.tensor_scalar(
                            out=et[:].bitcast(I8),
                            in0=sp[:],
                            scalar1=EXPA,
                            scalar2=EXPB,
                            op0=OP.mult,
                            op1=OP.add,
                        )
                    if fill is not None:
                        fill(8 * h + pair)
                    nc.tensor.matmul(
                        pc[0:65, :],
                        v8t[pair][:, :, h, :],
                        et[:],
                        start=(pair == 0),
                        stop=(pair == 7),
                        perf_mode=DR,
                    )
                    if norm_hook is not None:
                        norm_hook(pair)
                return pc

            # normalize: ctx8[d, i] = CTXSCALE * ctx~[d, i] / denom[i].
            # Split across the NEXT head's pair stream: reciprocal +
            # broadcast after pair 4, stt after pair 7 — the Pool
            # broadcast latency hides behind two more exps, so DVE never
            # stalls on it.
            def emit_recip(h, pc):
                rec = recp.tile([1, QR], F32, tag="rec", name=f"rc_{h}")
                nc.vector.reciprocal(out=rec, in_=pc[64:65, :])
                rb = recp.tile([64, QR], F32, tag="rb", name=f"rb_{h}")
                nc.gpsimd.partition_broadcast(rb[:], rec[:], channels=64)
                return rb

            def emit_stt(h, pc, rb):
                t, r0 = h // 2, 64 * (h % 2)
                nc.vector.scalar_tensor_tensor(
                    out=ctx8[r0 : r0 + 64, t, :],
                    in0=pc[0:64, :],
                    scalar=CTXSCALE,
                    in1=rb[:],
                    op0=OP.mult,
                    op1=OP.mult,
                )

            # fill schedule: (deadline pair-slot, kind, a, b). Slot s =
            # 8*h + pair. Deadlines are set just after the gating DMA's
            # estimated arrival so a late fill never blocks the in-order
            # PE/ACT streams.
            # Fill schedule. Superhead (heads 0/1) consumes slots 0-7 (one
            # per interleave step); the main loop uses slots 8*h + pair.
            # K tile 0's last chunks and K tile 1 land inside the superhead
            # scores phase (gated on the wk remainder + kc DMAs); V dh0 is
            # emitted inline in the superhead attn@V phase.
            fills = []
            fills.append((3, "KC", 0, 2))
            fills.append((4, "KC", 0, 3))
            fills.append((5, "K", 1, 0))
            fills.append((6, "K", 1, 1))
            fills.append((7, "QS", 1, 0))
            for jp in range(8):
                fills.append((16 + 5 * jp, "V", 1, jp))
            for t in range(2, 8):
                fills.append((16 * t - 12, "K", t, 0))
                fills.append((16 * t - 6, "K", t, 1))
            for tp in range(1, 4):
                fills.append((32 * tp - 8, "Q", tp, 0))
            fills.sort(key=lambda it: it[0])
            fill_state = {"i": 0}

            def emit_unit(kind, a, b2):
                if kind == "K":
                    emit_khalf(a, b2)
                elif kind == "KC":
                    emit_kchunk(a, b2)
                elif kind == "V":
                    emit_vpair(a, b2)
                elif kind == "QS":
                    emit_qsingle(a)
                else:
                    emit_qpair(a)

            def fill_one(s):
                while fill_state["i"] < len(fills) and fills[fill_state["i"]][0] <= s:
                    _, kind, a, b2 = fills[fill_state["i"]]
                    fill_state["i"] += 1
                    emit_unit(kind, a, b2)

            # ---- superhead: heads 0 and 1 interleaved ----
            # Both heads live in K/Q tile 0, so all 16 score pairs + exps
            # run during the input-DMA window; the attn@V accumulations
            # (gated on the v8 stream) trail behind with the V dh0
            # projections emitted inline.
            emit_qsingle(0)
            emit_kchunk(0, 0)
            emit_kchunk(0, 1)
            SH_ENG = {(0, 0): "A", (1, 0): "D", (0, 1): "D", (1, 1): "A"}
            ets01 = {}
            pc01 = {}
            for pair in range(8):
                for hh in (0, 1):
                    r0 = 64 * hh
                    et = expp.tile(
                        [128, 2, QR], F8E5, tag="exp", name=f"e_{hh}_{pair}"
                    )
                    ets01[(hh, pair)] = et
                    sp = psc.tile([128, 2, QR], F32, tag="sc", name=f"s_{hh}_{pair}")
                    for s in range(2):
                        jt = 2 * pair + s
                        nc.tensor.matmul(
                            sp[:, s, :],
                            KT8[r0 : r0 + 64, 0, jt, :]
                            .unsqueeze(1)
                            .broadcast_to([64, 2, 128]),
                            QT8[r0 : r0 + 64, 0, :, :],
                            start=True,
                            stop=True,
                            perf_mode=DR,
                        )
                    if SH_ENG[(hh, pair % 2)] == "A":
                        nc.scalar.activation(
                            out=et[:], in_=sp[:], func=AF.Exp, scale=0.125
                        )
                    else:
                        nc.vector.tensor_scalar(
                            out=et[:].bitcast(I8),
                            in0=sp[:],
                            scalar1=EXPA,
                            scalar2=EXPB,
                            op0=OP.mult,
                            op1=OP.add,
                        )
                fill_one(pair)
            pc01[0] = pcp.tile([128, QR], F32, tag="pc", name="pc_0")
            pc01[1] = pcp.tile([128, QR], F32, tag="pc", name="pc_1")
            for pair in range(8):
                emit_vpair(0, pair)
                for hh in (0, 1):
                    nc.tensor.matmul(
                        pc01[hh][0:65, :],
                        v8t[pair][:, :, hh, :],
                        ets01[(hh, pair)][:],
                        start=(pair == 0),
                        stop=(pair == 7),
                        perf_mode=DR,
                    )
            # normalize head 0 now (frees its PSUM bank before head 2)
            rb0 = emit_recip(0, pc01[0])
            emit_stt(0, pc01[0], rb0)

            prev = {"pc": pc01[1], "h": 1, "rb": None}

            def norm_hook(pair):
                if prev["pc"] is None:
                    return
                if pair == 4:
                    prev["rb"] = emit_recip(prev["h"], prev["pc"])
                elif pair == 7:
                    emit_stt(prev["h"], prev["pc"], prev["rb"])
                    prev["pc"] = None

            for h in range(2, 16):
                pc = emit_head(h, fill=fill_one, norm_hook=norm_hook)
                prev["pc"], prev["h"] = pc, h
            rb15 = emit_recip(15, prev["pc"])
            emit_stt(15, prev["pc"], rb15)
            # drain any leftover fills
            while fill_state["i"] < len(fills):
                _, kind, a, b2 = fills[fill_state["i"]]
                fill_state["i"] += 1
                emit_unit(kind, a, b2)

            # ------------- out projection + residual + LayerNorm -------------
            # Residual folds into the out-proj PSUM group via a 256*I
            # matmul, so PSUM holds 256*x; LN is scale-invariant once eps
            # is scaled by 256^2. Sum(x) comes from 9 tiny extra matmuls
            # (host-precomputed Wo column-sums + resid row-sums), Sum(x^2)
            # from an ACT Square pass with the free-axis accumulator —
            # bn_stats never touches DVE. rstd via DVE rsqrt bit-trick +
            # one Newton step; the (x-mu)*rstd affine is a single ACT
            # Identity(scale, bias) over both PSUM banks.
            MAGIC = float(0x5F3759DF)
            EPS256 = EPS * 65536.0
            po2s, sums, sx2s, ws = {}, {}, {}, {}

            def emit_po(qb):
                po2 = psc.tile([128, 2, QR], F32, tag="sc", name=f"po_{qb}")
                po2s[qb] = po2
                for eh in range(2):
                    po = po2[:, eh, :]
                    for p in range(4):
                        nc.tensor.matmul(
                            po[:],
                            ctx8[:, 2 * p : 2 * p + 2, qb * 128 : (qb + 1) * 128],
                            wov[:, 2 * p : 2 * p + 2, eh * 512 : (eh + 1) * 512],
                            start=(p == 0),
                            stop=False,
                            perf_mode=DR,
                        )
                    nc.tensor.matmul(
                        po,
                        id_t[:],
                        res_t[qb][:, eh * 512 : (eh + 1) * 512],
                        start=False,
                        stop=True,
                    )
                # 256*Sum(x) into one PSUM column
                sm = pcp.tile([128, QR], F32, tag="pc", name=f"sm_{qb}")
                sums[qb] = sm
                for sl in range(8):
                    nc.tensor.matmul(
                        sm[:, 0:1],
                        ctx8[:, sl, qb * 128 : (qb + 1) * 128],
                        csb[:, sl : sl + 1],
                        start=(sl == 0),
                        stop=False,
                    )
                nc.tensor.matmul(
                    sm[:, 0:1],
                    id_t[:],
                    rs_t[:, qb : qb + 1],
                    start=False,
                    stop=True,
                )

            def emit_square(qb):
                sq = lnp.tile([128, E], BF16, tag="sq", name=f"sq_{qb}")
                sx2 = lnp.tile([128, 1], F32, tag="sx2", name=f"sx2_{qb}", bufs=4)
                sx2s[qb] = sx2
                nc.scalar.activation(
                    out=sq[:].rearrange("p (s c) -> p s c", s=2),
                    in_=po2s[qb][:],
                    func=AF.Square,
                    accum_out=sx2[:, 0:1],
                )

            def emit_wchain(qb):
                # w cols: 0=mu(256x), 1=mu^2-eps256, 2=var+eps, 3=y0(bit),
                # 4=scratch, 5=rstd(256x), 6=-mu*rstd
                w = lnp.tile([128, 8], F32, tag="w", name=f"w_{qb}", bufs=4)
                ws[qb] = w
                nc.vector.tensor_scalar(
                    out=w[:, 0:1], in0=sums[qb][:, 0:1],
                    scalar1=1.0 / 1024.0, scalar2=None, op0=OP.mult,
                )
                nc.vector.tensor_scalar(
                    out=w[:, 1:2], in0=w[:, 0:1],
                    scalar1=w[:, 0:1], scalar2=-EPS256, op0=OP.mult, op1=OP.add,
                )
                nc.vector.scalar_tensor_tensor(
                    out=w[:, 2:3], in0=sx2s[qb][:, 0:1],
                    scalar=1.0 / 1024.0, in1=w[:, 1:2],
                    op0=OP.mult, op1=OP.subtract,
                )
                nc.vector.tensor_scalar(
                    out=w[:, 3:4].bitcast(mybir.dt.int32),
                    in0=w[:, 2:3].bitcast(mybir.dt.int32),
                    scalar1=-0.5, scalar2=MAGIC, op0=OP.mult, op1=OP.add,
                )
                nc.vector.tensor_tensor(
                    out=w[:, 4:5], in0=w[:, 3:4], in1=w[:, 3:4], op=OP.mult
                )
                nc.vector.tensor_tensor(
                    out=w[:, 4:5], in0=w[:, 4:5], in1=w[:, 2:3], op=OP.mult
                )
                nc.vector.tensor_scalar(
                    out=w[:, 4:5], in0=w[:, 4:5],
                    scalar1=-0.5, scalar2=1.5, op0=OP.mult, op1=OP.add,
                )
                nc.vector.tensor_tensor(
                    out=w[:, 5:6], in0=w[:, 3:4], in1=w[:, 4:5], op=OP.mult
                )
                nc.vector.tensor_scalar(
                    out=w[:, 6:7], in0=w[:, 0:1],
                    scalar1=w[:, 5:6], scalar2=-1.0, op0=OP.mult, op1=OP.mult,
                )

            def emit_ln_out(qb):
                w = ws[qb]
                a = lnp.tile([128, E], BF16, tag="a", name=f"a_{qb}")
                nc.scalar.activation(
                    out=a[:],
                    in_=po2s[qb][:],
                    func=AF.Identity,
                    bias=w[:, 6:7],
                    scale=w[:, 5:6],
                )
                y = lnp.tile([128, E], BF16, tag="y", name=f"y_{qb}")
                nc.vector.tensor_tensor(out=y, in0=a, in1=gamma_b, op=OP.mult)
                nc.vector.tensor_tensor(out=y, in0=y, in1=beta_b, op=OP.add)
                nc.sync.dma_start(
                    out=out[qb * 128 : (qb + 1) * 128, :], in_=y
                )

            # software-pipelined emission: Square(qb+1) lands on ACT before
            # affine(qb) so ACT never waits on the DVE w-chain.
            emit_po(0)
            emit_square(0)
            emit_po(1)
            emit_square(1)
            emit_wchain(0)
            emit_po(2)
            emit_square(2)
            emit_ln_out(0)
            emit_wchain(1)
            emit_po(3)
            emit_square(3)
            emit_ln_out(1)
            emit_wchain(2)
            emit_ln_out(2)
            emit_wchain(3)
            emit_ln_out(3)

    nc.finalize()
    return nc


def _interleave_etiles(arr):
    """[E, N] -> [128, 8*N] with e = slot*128 + partition pairing layout."""
    Edim, N = arr.shape
    return np.ascontiguousarray(
        arr.reshape(8, 128, N).transpose(1, 0, 2).reshape(128, 8 * N)
    )


def build_in_maps(inputs):
    q = np.asarray(inputs["query"], dtype=np.float32)
    k = np.asarray(inputs["key"], dtype=np.float32)
    v = np.asarray(inputs["value"], dtype=np.float32)
    Wq = np.asarray(inputs["Wq"], dtype=np.float32)
    bq = np.asarray(inputs["bq"], dtype=np.float32)
    Wk = np.asarray(inputs["Wk"], dtype=np.float32)
    Wv = np.asarray(inputs["Wv"], dtype=np.float32)
    bv = np.asarray(inputs["bv"], dtype=np.float32)
    Wo = np.asarray(inputs["Wo"], dtype=np.float32)
    bo = np.asarray(inputs["bo"], dtype=np.float32)
    gamma = np.asarray(inputs["gamma"], dtype=np.float32)
    beta = np.asarray(inputs["beta"], dtype=np.float32)

    e4 = ml_dtypes.float8_e4m3
    # weights: pre-scaled x32, e = slot*128 + partition layout
    wq8 = _interleave_etiles(Wq.T * WSCALE).astype(e4)
    wk8 = _interleave_etiles(Wk.T * WSCALE).astype(e4)
    wv8 = _interleave_etiles(Wv.T * WSCALE).astype(e4)
    wo8 = _interleave_etiles(Wo.T * WSCALE).astype(e4)

    # per-batch activations
    k8 = [_interleave_etiles(np.ascontiguousarray(k[b].T)).astype(e4) for b in range(B)]
    v8 = []
    for b in range(B):
        t = _interleave_etiles(np.ascontiguousarray(v[b].T))  # [128, 8*2048]
        t = (
            t.reshape(128, 8, 16, 128)
            .transpose(0, 2, 1, 3)
            .reshape(128, 16 * 8 * 128)
        )
        v8.append(np.ascontiguousarray(t).astype(e4))

    # bv folded into a host-side bias vector: out includes +bv @ Wo.T + bo.
    bo_eff = (bv @ Wo.T + bo).astype(np.float32)
    # column sums of the (scaled, transposed) out-proj weights, used by the
    # device to compute Sum_e(out) with tiny matmuls (e = slot*128 + p)
    cs_vec = (Wo.sum(axis=0) * WSCALE).astype(np.float32)
    cs8 = np.ascontiguousarray(cs_vec.reshape(8, 128).T).astype(e4)

    in_maps = []
    for c in range(N_CORES):
        b, g = divmod(c, 2)
        rows = slice(QR * g, QR * g + QR)
        q8 = _interleave_etiles(np.ascontiguousarray(q[b, rows, :].T)).astype(e4)
        resid_bf = np.ascontiguousarray(q[b, rows, :] + bo_eff).astype(
            ml_dtypes.bfloat16
        )
        rs = resid_bf.astype(np.float32).sum(axis=1)  # [512]
        in_maps.append(
            {
                "q8": q8,
                "k8": k8[b],
                "v8": v8[b],
                "wq8": wq8,
                "wk8": wk8,
                "wv8": wv8,
                "wo8": wo8,
                "bq8": np.ascontiguousarray(bq.reshape(8, 128).T),
                "cs8": cs8,
                "rs4": np.ascontiguousarray(rs.reshape(4, 128).T).astype(
                    ml_dtypes.bfloat16
                ),
                "resid": resid_bf,
                "ident": (np.eye(128, dtype=np.float32) * (WSCALE * CTXSCALE)).astype(
                    ml_dtypes.bfloat16
                ),
                "vec3": np.ascontiguousarray(np.stack([gamma, beta])).astype(
                    ml_dtypes.bfloat16
                ),
            }
        )
    return in_maps


def kernel(**inputs):
    global _NC_CACHE, LAST_RESULTS
    if _NC_CACHE is None:
        _NC_CACHE = _build_nc()
    nc = _NC_CACHE

    in_maps = build_in_maps(inputs)

    res = run_bass_kernel_spmd(nc, in_maps, list(range(N_CORES)), trace=TRACE)
    LAST_RESULTS = res

    outp = np.empty((B, SQ, E), dtype=np.float32)
    for c in range(N_CORES):
        b, g = divmod(c, 2)
        outp[b, QR * g : QR * g + QR, :] = res.results[c]["out"].astype(np.float32)
    return outp
